# revision 16
# baseline (speedup 1.0000x reference)
"""Trainium2 Bass kernel for nn_CrossAttentionModule_bias (V3).

Math (B=2, C=256, H=W=64, N=4096):
    q = queries.reshape(B,C,N).T + q_pos        # [B,N,C]
    k = keys.reshape(B,C,N).T + k_pos
    v = values.reshape(B,C,N).T
    attn = softmax(q @ k.T / sqrt(C)) + c_b     # c_b: per-batch SCALAR
    out  = attn @ v   -> [B,C,H,W]

c_b is a per-batch scalar, so the post-softmax "+ c_b" adds the rank-1 term
c_b * colsum(V) to every output row; that term is applied on the HOST after
the gather.  The device computes the pure softmax attention.

Device kernel (per core, 8 cores = 2 batches x 4 query shards of 1024),
per (n-tile nt of 512 queries, m-chunk mc of 128 keys):
    dots[m,n] = sum_c keff[c,m] * qeff[c,n]     (2 bf16 matmuls, dots PSUM)
    expt[m,n] = exp(dots * 1/16)                (ACT, bf16 out, no max-sub)
    u[c,n]   += vt[m-chunk,c-chunk]^T @ expt    (2 bf16 matmuls, u PSUM)
    zacc[m%128, n] += expt                      (DVE partial colsums, fp32)
tail per nt (overlaps the next n-tile's m-loop):
    z[1,n] = ones^T @ zacc                      (1 matmul)
    recip = 1/z (DVE) ; bc[c,n] = ones_col @ recip (1 bf16 matmul, K=1)
    out[c,n] = u * bc  (DVE) -> DMA

PE work: 128 QK + 128 AV matmuls of N=512 + 4 tiny tail matmuls; the
exp round-trip and all DVE work hide under the PE stream (QK runs
`lookahead` steps ahead in a 3-deep dots ring).
"""

import numpy as np
import ml_dtypes

import concourse.bass as bass
import concourse.mybir as mybir
import concourse.tile as tile
from concourse import bacc
from concourse.bass_utils import run_bass_kernel_spmd

# Problem shape (hardcoded per the task contract)
B, C, H, W = 2, 256, 64, 64
N = H * W                      # 4096
NCORES = 8
SHARDS_PER_B = NCORES // B     # 4 query shards per batch
NSH = N // SHARDS_PER_B        # 1024 query rows per core
SCALE = float(C) ** -0.5       # 1/16
P = 128
CCN = C // P                   # 2 c-chunks
MCN = N // P                   # 32 m-chunks
NT_SIZE = 512                  # n-tile (psum-bank fp32 limit)
NTN = NSH // NT_SIZE           # 2 n-tiles per core

F32 = mybir.dt.float32
F32R = mybir.dt.float32r
BF16 = mybir.dt.bfloat16
NPBF16 = ml_dtypes.bfloat16

EXP = mybir.ActivationFunctionType.Exp

_CACHE: dict = {}


def _build_bass(reps: int = 1, loop_reps: int = 0, ablate: tuple = (), lookahead: int = 2):
    """loop_reps>0 wraps the compute in a hardware For_i loop (timing: slope
    between two loop_reps builds isolates per-iteration HW time); with
    loop_reps, `reps` bodies are emitted per loop iteration."""
    nc = bacc.Bacc("TRN2", target_bir_lowering=False, debug=False)

    keff = nc.dram_tensor("keff", [C, N], BF16, kind="ExternalInput")
    qeff = nc.dram_tensor("qeff", [C, NSH], BF16, kind="ExternalInput")
    vt = nc.dram_tensor("vt", [N, C], BF16, kind="ExternalInput")
    out = nc.dram_tensor("out", [C, NSH], F32, kind="ExternalOutput")

    KQ = 8                     # m-chunks per keff DMA tile
    KQN = MCN // KQ            # 4 keff tiles per c-chunk

    with tile.TileContext(nc) as tc:
        with (
            tc.tile_pool(name="const", bufs=1) as cpool,
            tc.tile_pool(name="work", bufs=4) as wpool,
            tc.tile_pool(name="zacc", bufs=2) as zpool,
            tc.tile_pool(name="tail", bufs=2) as tpool,
            tc.tile_pool(name="dots_ps", bufs=3, space="PSUM") as dots_pool,
            tc.tile_pool(name="acc_ps", bufs=1, space="PSUM") as acc_pool,
            tc.tile_pool(name="zb_ps", bufs=1, space="PSUM") as zb_pool,
        ):
            zero = cpool.tile([P, 1], F32, tag="zero", name="zero")
            nc.vector.memset(zero[:], 0.0)
            ones_col = cpool.tile([P, 1], BF16, tag="ones_col", name="ones_col")
            nc.vector.memset(ones_col[:], 1.0)
            ones_row = cpool.tile([1, P], BF16, tag="ones_row", name="ones_row")
            nc.vector.memset(ones_row[:], 1.0)

            qeff_t = []
            for cc in range(CCN):
                t = cpool.tile([P, NSH], BF16, tag=f"qeff{cc}", name=f"qeff{cc}")
                nc.sync.dma_start(t[:], qeff[cc * P : (cc + 1) * P, :])
                qeff_t.append(t)

            # keff split into [128, KQ*128] tiles so QK can start early
            keff_t = [[None] * KQN for _ in range(CCN)]
            for q in range(KQN):
                for cc in range(CCN):
                    t = cpool.tile([P, KQ * P], BF16, tag=f"keff{cc}_{q}", name=f"keff{cc}_{q}")
                    nc.sync.dma_start(
                        t[:], keff[cc * P : (cc + 1) * P, q * KQ * P : (q + 1) * KQ * P]
                    )
                    keff_t[cc][q] = t

            vt_t = []
            for mc in range(MCN):
                t = cpool.tile([P, C], BF16, tag=f"vt{mc}", name=f"vt{mc}")
                nc.sync.dma_start(t[:], vt[mc * P : (mc + 1) * P, :])
                vt_t.append(t)

            const_expt = None
            if "exp" in ablate or "qk" in ablate:
                const_expt = cpool.tile([P, NT_SIZE], BF16, tag="cexpt", name="cexpt")
                nc.vector.memset(const_expt[:], 1.0)

            def emit_qk(nt, mc):
                ns = slice(nt * NT_SIZE, (nt + 1) * NT_SIZE)
                dots = dots_pool.tile([P, NT_SIZE], F32, tag="dots", name="dots")
                for cc in range(CCN):
                    lhsT = keff_t[cc][mc // KQ][:, (mc % KQ) * P : (mc % KQ + 1) * P]
                    nc.tensor.matmul(
                        dots[:],
                        lhsT,
                        qeff_t[cc][:, ns],
                        start=(cc == 0),
                        stop=(cc == CCN - 1),
                    )
                return dots

            def emit_body():
                skip_qk = "qk" in ablate
                skip_av = "av" in ablate
                skip_z = "z" in ablate

                def alloc_acc(nt):
                    if skip_av:
                        return None
                    return [
                        acc_pool.tile([P, NT_SIZE], F32, tag=f"u{cc}n{nt}", name=f"u{cc}n{nt}")
                        for cc in range(CCN)
                    ]

                u_ps = [None] * NTN
                u_ps[0] = alloc_acc(0)
                zacc = [None] * NTN

                def emit_tail(nt):
                    # z reduce + 1/z partition-broadcast + u*(1/z); overlaps
                    # the next n-tile's m-loop
                    ns = slice(nt * NT_SIZE, (nt + 1) * NT_SIZE)
                    with nc.allow_low_precision(reason="bf16 z-reduce: 0.4% on 1/Z, budget 2e-2"):
                        zacc_b = tpool.tile([P, NT_SIZE], BF16, tag="zacc_b", name="zacc_b")
                        nc.vector.tensor_copy(zacc_b[:], zacc[nt][:])
                        zb = zb_pool.tile([P, NT_SIZE], F32, tag="zb", name="zb")
                        nc.tensor.matmul(zb[0:1, :], ones_col[:], zacc_b[:], start=True, stop=True)
                        recip = tpool.tile([1, NT_SIZE], BF16, tag="recip", name="recip")
                        nc.vector.reciprocal(recip[:], zb[0:1, :])
                    nc.tensor.matmul(zb[:], ones_row[:], recip[:], start=True, stop=True)
                    bc_sb = tpool.tile([P, NT_SIZE], F32, tag="bc_sb", name="bc_sb")
                    nc.vector.tensor_copy(bc_sb[:], zb[:])
                    for cc in range(CCN):
                        outsb = tpool.tile([P, NT_SIZE], F32, tag="outsb", name="outsb")
                        nc.vector.tensor_mul(outsb[:], u_ps[nt][cc][:], bc_sb[:])
                        nc.sync.dma_start(out[cc * P : (cc + 1) * P, ns], outsb[:])

                # software-pipelined: QK runs `lookahead` steps ahead so PE
                # never waits on the ACT exp round-trip
                steps = [(nt, mc) for nt in range(NTN) for mc in range(MCN)]
                dots_q = [] if skip_qk else [emit_qk(*steps[i]) for i in range(lookahead)]
                for i, (nt, mc) in enumerate(steps):
                    if skip_qk:
                        expt = const_expt
                    else:
                        dots = dots_q.pop(0)
                        if "exp" in ablate:
                            expt = const_expt
                        else:
                            expt = wpool.tile([P, NT_SIZE], BF16, tag="expt", name="expt")
                            nc.scalar.activation(expt[:], dots[:], EXP, bias=zero[:], scale=SCALE)
                        if i + lookahead < len(steps):
                            dots_q.append(emit_qk(*steps[i + lookahead]))
                    first, last = mc == 0, mc == MCN - 1
                    if not skip_av:
                        for cc in range(CCN):
                            nc.tensor.matmul(
                                u_ps[nt][cc][:],
                                vt_t[mc][:, cc * P : (cc + 1) * P],
                                expt[:],
                                start=first,
                                stop=last,
                            )
                    if not skip_z:
                        if first:
                            zacc[nt] = zpool.tile([P, NT_SIZE], F32, tag="zacc", name="zacc")
                            nc.vector.tensor_copy(zacc[nt][:], expt[:])
                        else:
                            nc.vector.tensor_add(zacc[nt][:], zacc[nt][:], expt[:])
                    if last and not skip_av and not skip_z and "tail" not in ablate:
                        emit_tail(nt)
                    if last and nt + 1 < NTN:
                        u_ps[nt + 1] = alloc_acc(nt + 1)

            if loop_reps > 0:
                with tc.For_i(0, loop_reps, 1, hint_engines=(mybir.EngineType.PE,)):
                    for _ in range(reps):
                        emit_body()
            else:
                for _ in range(reps):
                    emit_body()

    nc.compile()
    return nc


def _prep_inputs(queries, keys, values, mask_eye, mask_mouth, q_pos, k_pos,
                 bias_eye, bias_mouth):
    """Host-side shard prep: positional adds, V transpose (bf16), plus the
    rank-1 post-softmax term c_b * colsum(V) applied after the gather."""
    q = queries.reshape(B, C, N) + q_pos[0].T[None]
    k = keys.reshape(B, C, N) + k_pos[0].T[None]
    vT = np.ascontiguousarray(values.reshape(B, C, N).transpose(0, 2, 1))  # [B,N,C]

    def msum(mask):
        # nearest resize 128->64 picks every other row/col
        m = mask[:, :, ::2, ::2].reshape(B, -1)
        return (m * m).sum(axis=1, dtype=np.float64)

    softplus = lambda x: np.logaddexp(0.0, x)
    c_b = softplus(float(bias_eye[0]) * msum(mask_eye)) + softplus(
        float(bias_mouth[0]) * msum(mask_mouth)
    )  # [B]
    S = vT.sum(axis=1, dtype=np.float64)  # [B, C]
    cbs = (c_b[:, None] * S).astype(np.float32)  # [B, C]

    kb = k.astype(NPBF16)
    qb = q.astype(NPBF16)
    vb = vT.astype(NPBF16)
    in_maps = []
    for core in range(NCORES):
        b, sh = divmod(core, SHARDS_PER_B)
        n0 = sh * NSH
        in_maps.append(
            {
                "keff": np.ascontiguousarray(kb[b]),
                "qeff": np.ascontiguousarray(qb[b][:, n0 : n0 + NSH]),
                "vt": vb[b],
            }
        )
    return in_maps, cbs


def kernel(**inputs) -> np.ndarray:
    inputs = {k: np.asarray(v, np.float32) for k, v in inputs.items()}
    in_maps, cbs = _prep_inputs(**inputs)

    if "nc" not in _CACHE:
        _CACHE["nc"] = _build_bass()
    res = run_bass_kernel_spmd(_CACHE["nc"], in_maps, list(range(NCORES)))

    full = np.empty((B, C, N), np.float32)
    for core in range(NCORES):
        b, sh = divmod(core, SHARDS_PER_B)
        n0 = sh * NSH
        full[b][:, n0 : n0 + NSH] = res.results[core]["out"]
    full += cbs[:, :, None]
    return full.reshape(B, C, H, W)
